# revision 3
# baseline (speedup 1.0000x reference)
"""BjorckLinear TRN2 kernel v2 (8-core SPMD, data-parallel over batch).

reference semantics:
    w10 = bjorck_orthonormalize(weight)   # exactly 10 order-1 iterations
    out = inputs @ w10.T

v2 changes vs baseline:
  - X shard is passed host-side as bf16 [512, 16384] and loaded WHOLE
    into SBUF (128KB/partition) during the Bjorck phase, so the linear
    phase never waits on input DMA.
  - V10 (= W10^T) is cast to bf16 at eviction; the big linear runs
    bf16 x bf16 (same 1 cycle/row PE rate as f32r, half the SBUF/DMA).
  - Y is evicted to bf16 and DMA'd out as bf16 [512, 16384]; host
    upcasts. Total HBM traffic drops 67MB -> 34MB per core, moving the
    kernel from DMA-bound to PE-bound.
  - Bjorck itself stays f32r (precision for the 10-iteration recurrence).

Device algorithm per core:
    iterate W (with WT = W^T maintained via PE transposes):
        S = W^T W               (lhsT = W chunks, rhs = W)
        G = S - 3I              (DVE/ACT eviction + diagonal-block subtract)
        W' = -0.5 * (W G)       (lhsT = WT, rhs = G; -0.5 in the eviction)
        WT' = transpose(W')     (PE transpose, 128x128 blocks)
    last iteration computes only V10 = W10^T = -0.5 * (G @ WT) via
    lhsT = G (G symmetric), evicted to bf16.
    Then Yt = W10 @ Xt with lhsT = V10 chunks, rhs = resident X tiles.
"""
import numpy as np
import ml_dtypes

import concourse.bacc as bacc
import concourse.mybir as mybir
import concourse.tile as tile
from concourse.bass_utils import run_bass_kernel_spmd

dt = mybir.dt

P = 128
D = 512
KC = D // P            # 4 contraction chunks
ITERS = 10
N_CORES = 8
BATCH = 131072
SHARD = BATCH // N_CORES   # 16384

YBLK = 2048
NYB = SHARD // YBLK    # 8 output col-blocks
NSUB = YBLK // 512     # 4 psum sub-blocks per output block
YBUFS = 4

PSUM_TAGS = ["pa", "pb", "pc", "pd"]


def build():
    nc = bacc.Bacc("TRN2", target_bir_lowering=False, debug=False)
    xt_dram = nc.dram_tensor("xt", [D, SHARD], dt.bfloat16, kind="ExternalInput")
    # One contiguous pack per ring so each is a single fast DMA:
    #  wq  = [W row-chunks concat | 3*I_128 | I_128]  (sync ring)
    #  wtq = [W^T row-chunks concat]                  (scalar ring)
    wq_dram = nc.dram_tensor("wq", [P, KC * D + 2 * P], dt.float32r,
                             kind="ExternalInput")
    wtq_dram = nc.dram_tensor("wtq", [P, KC * D], dt.float32r,
                              kind="ExternalInput")
    yt_dram = nc.dram_tensor("yt", [D, SHARD], dt.bfloat16, kind="ExternalOutput")

    with tile.TileContext(nc) as tc:
        with (
            tc.tile_pool(name="const", bufs=1) as const,
            tc.tile_pool(name="bj", bufs=2) as bj,
            tc.tile_pool(name="gp", bufs=1) as gp,
            tc.tile_pool(name="xfull", bufs=1) as xfull,
            tc.tile_pool(name="yp", bufs=YBUFS) as yp,
            tc.tile_pool(name="psum", bufs=2, space="PSUM") as psum,
        ):
            # ---------- loads ----------
            # sync ring: the W pack (one DMA, Bjorck-critical).
            # scalar ring: the WT pack (then it is free until y-out).
            wq_t = const.tile([P, KC * D + 2 * P], dt.float32r, tag="wq")
            nc.sync.dma_start(wq_t[:], wq_dram[:, :])
            wtq_t = const.tile([P, KC * D], dt.float32r, tag="wtq")
            nc.scalar.dma_start(wtq_t[:], wtq_dram[:, :])
            W = [wq_t[:, k * D:(k + 1) * D] for k in range(KC)]
            WT = [wtq_t[:, k * D:(k + 1) * D] for k in range(KC)]
            e128 = wq_t[:, KC * D:KC * D + P]
            i128 = wq_t[:, KC * D + P:KC * D + 2 * P]
            # X loads go on the gpsimd ring in 0.5MB chunks: the tile
            # scheduler's periodic sem checkpoints can then only ever block
            # an engine ~1.5us (one chunk), not ~12us (a whole 4MB tile).
            # A tiny gpsimd read of the W/WT tiles first makes the whole X
            # stream (FIFO behind it) wait until the Bjorck-critical weight
            # loads have landed, so X doesn't steal their HBM bandwidth.
            X = []
            for k in range(KC):
                xk = xfull.tile([P, SHARD], dt.bfloat16, tag=f"x_{k}")
                X.append(xk)
            wgate = const.tile([P, 8], dt.float32r, tag="wgate")
            nc.gpsimd.tensor_tensor(wgate[:], W[KC - 1][:, 0:8],
                                    WT[KC - 1][:, 0:8], mybir.AluOpType.add)

            # ---------- PE warm-up ----------
            # The PE would otherwise idle until the W pack lands (~14us), so
            # the first ~16 real matmuls would run at the HAM-throttled
            # 1.2 GHz. 12 dummy matmuls on a zeroed tile (DVE queue boots
            # ~7us, each cold mm ~427ns -> ends ~13us, right at W arrival)
            # get HAM to K=8/8 beforehand. Even count per PSUM tag keeps
            # the buffer-rotation parity; the dummy banks are never read.
            warm = const.tile([P, D], dt.bfloat16, tag="warm")
            nc.vector.memset(warm[:], 0)
            for wi in range(12):
                wps = psum.tile([P, D], dt.float32,
                                tag="pa" if wi % 2 == 0 else "pb",
                                name=f"ps_warm_{wi}")
                nc.tensor.matmul(wps[:], warm[:, 0:P], warm[:],
                                 start=True, stop=True)
            for nb in range(NYB):
                csl = slice(nb * YBLK, (nb + 1) * YBLK)
                for k in range(KC):
                    nc.gpsimd.dma_start(X[k][:, csl],
                                        xt_dram[k * P:(k + 1) * P, csl])

            # ---------- Bjorck (replicated) ----------
            # PSUM tags (bufs=2 each, 8 banks total):
            #   S groups cycle "pa" only (reuse distance 2 groups; the bank
            #   frees after the ACT/DVE copy alone since the diagonal
            #   subtract reads the SBUF copy).
            #   W' groups alternate "pb"/"pc" (reuse distance >= 1 iter).
            #   Transposes (and V10 on the last iter) use "pd"+"pc": all 4
            #   tps tiles live at once so transposes run SUB-major and never
            #   wait on the last W' eviction.
            # Latency-critical last evictions (G[3], W'[3]) are split in
            # halves across ACT and DVE so their consumers don't stall.
            V10 = []
            for it in range(ITERS):
                last = it == ITERS - 1
                # S = W^T W ; G = S - 3I
                G = []
                for mi in range(KC):
                    msl = slice(mi * P, (mi + 1) * P)
                    ps = psum.tile([P, D], dt.float32, tag="pa",
                                   name=f"ps_s_{it}_{mi}")
                    for ki in range(KC):
                        nc.tensor.matmul(ps[:], W[ki][:, msl], W[ki][:],
                                         start=(ki == 0), stop=(ki == KC - 1))
                    g = gp.tile([P, D], dt.float32r, tag=f"g_{mi}")
                    if mi == 3:
                        nc.scalar.copy(g[:, 0:D // 2], ps[:, 0:D // 2])
                        nc.vector.tensor_copy(g[:, D // 2:D], ps[:, D // 2:D])
                    elif mi % 2 == 0:
                        nc.scalar.copy(g[:], ps[:])
                    else:
                        nc.vector.tensor_copy(g[:], ps[:])
                    # diagonal block: G[:, msl] -= 3I (reads the SBUF copy,
                    # not PSUM, so the bank frees as soon as the copy is done)
                    nc.vector.tensor_tensor(g[:, msl], g[:, msl], e128[:],
                                            mybir.AluOpType.subtract)
                    G.append(g)

                if last:
                    # V10 = W10^T = -0.5 * (G @ WT)  (lhsT = G, G symmetric);
                    # evicted directly to bf16 for the linear phase.
                    for mi in range(KC):
                        msl = slice(mi * P, (mi + 1) * P)
                        ps = psum.tile([P, D], dt.float32,
                                       tag="pd" if mi % 2 == 0 else "pc",
                                       name=f"ps_v10_{mi}")
                        for ki in range(KC):
                            nc.tensor.matmul(ps[:], G[ki][:, msl], WT[ki][:],
                                             start=(ki == 0),
                                             stop=(ki == KC - 1))
                        vt = const.tile([P, D], dt.bfloat16, tag=f"v10_{mi}")
                        if mi % 2 == 0:
                            nc.scalar.mul(vt[:], ps[:], -0.5)
                        else:
                            nc.vector.tensor_scalar_mul(vt[:], ps[:], -0.5)
                        V10.append(vt)
                    break

                # W' = -0.5 * (W G), lhsT = WT
                newW = []
                for mi in range(KC):
                    msl = slice(mi * P, (mi + 1) * P)
                    ps = psum.tile([P, D], dt.float32,
                                   tag="pb" if mi % 2 == 0 else "pc",
                                   name=f"ps_w_{it}_{mi}")
                    for ki in range(KC):
                        nc.tensor.matmul(ps[:], WT[ki][:, msl], G[ki][:],
                                         start=(ki == 0), stop=(ki == KC - 1))
                    wn = bj.tile([P, D], dt.float32r, tag=f"w_{mi}")
                    if mi == 3:
                        nc.scalar.mul(wn[:, 0:D // 2], ps[:, 0:D // 2], -0.5)
                        nc.vector.tensor_scalar_mul(
                            wn[:, D // 2:D], ps[:, D // 2:D], -0.5)
                    elif mi % 2 == 0:
                        nc.scalar.mul(wn[:], ps[:], -0.5)
                    else:
                        nc.vector.tensor_scalar_mul(wn[:], ps[:], -0.5)
                    newW.append(wn)

                # WT' = transpose(W') via PE, SUB-major: the first
                # transposes only need newW[0], so the PE never waits
                # on the last W' eviction.
                TPS = [psum.tile([P, D], dt.float32r,
                                 tag="pd" if mi % 2 == 0 else "pc",
                                 name=f"ps_t_{it}_{mi}")
                       for mi in range(KC)]
                for sub in range(KC):
                    ssl = slice(sub * P, (sub + 1) * P)
                    for mi in range(KC):
                        nc.tensor.transpose(TPS[mi][:, ssl],
                                            newW[sub][:, mi * P:(mi + 1) * P],
                                            i128[:])
                newWT = []
                for mi in range(KC):
                    vt = bj.tile([P, D], dt.float32r, tag=f"wt_{mi}")
                    if mi % 2 == 0:
                        nc.scalar.copy(vt[:], TPS[mi][:])
                    else:
                        nc.vector.tensor_copy(vt[:], TPS[mi][:])
                    newWT.append(vt)
                W, WT = newW, newWT

            # ---------- linear: Yt = W10 @ Xt  (lhsT = V10, X resident) ----
            for nb in range(NYB):
                for mi in range(KC):
                    msl = slice(mi * P, (mi + 1) * P)
                    PS = [psum.tile([P, 512], dt.float32, tag=PSUM_TAGS[js],
                                    name=f"ps_y_{nb}_{mi}_{js}")
                          for js in range(NSUB)]
                    yt = yp.tile([P, YBLK], dt.bfloat16, tag="y",
                                 name=f"y_{nb}_{mi}")
                    for ki in range(KC):
                        for js in range(NSUB):
                            c0 = nb * YBLK + js * 512
                            nc.tensor.matmul(
                                PS[js][:], V10[ki][:, msl],
                                X[ki][:, c0:c0 + 512],
                                start=(ki == 0), stop=(ki == KC - 1))
                    for js in range(NSUB):
                        osl = slice(js * 512, (js + 1) * 512)
                        if js < 2:
                            nc.scalar.copy(yt[:, osl], PS[js][:])
                        else:
                            nc.vector.tensor_copy(yt[:, osl], PS[js][:])
                    # y-out (512KB bf16) on the scalar ring (idle otherwise)
                    nc.scalar.dma_start(
                        yt_dram[mi * P:(mi + 1) * P,
                                nb * YBLK:(nb + 1) * YBLK], yt[:])
    nc.compile()
    return nc


_CACHE = {}


def _get_nc():
    if "nc" not in _CACHE:
        _CACHE["nc"] = build()
    return _CACHE["nc"]


def make_in_maps(inputs, weight):
    w = np.asarray(weight, dtype=np.float32)
    wt = w.T
    # wq = [W row-chunks | 3I | I] as one [128, 2304] pack; wtq likewise.
    wq = np.concatenate(
        [w[k * P:(k + 1) * P, :] for k in range(KC)]
        + [(3.0 * np.eye(P)).astype(np.float32), np.eye(P, dtype=np.float32)],
        axis=1)
    wq = np.ascontiguousarray(wq)
    wtq = np.ascontiguousarray(np.concatenate(
        [wt[k * P:(k + 1) * P, :] for k in range(KC)], axis=1))
    x = np.asarray(inputs, dtype=np.float32)
    in_maps = []
    for c in range(N_CORES):
        xt_c = x[c * SHARD:(c + 1) * SHARD, :].T.astype(ml_dtypes.bfloat16)
        in_maps.append({"xt": xt_c, "wq": wq, "wtq": wtq})
    return in_maps


def assemble_output(results) -> np.ndarray:
    out = np.empty((BATCH, D), dtype=np.float32)
    for c in range(N_CORES):
        out[c * SHARD:(c + 1) * SHARD, :] = \
            results[c]["yt"].astype(np.float32).T
    return out


def kernel(inputs: np.ndarray, weight: np.ndarray) -> np.ndarray:
    assert inputs.shape == (BATCH, D) and weight.shape == (D, D)
    nc = _get_nc()
    in_maps = make_in_maps(inputs, weight)
    res = run_bass_kernel_spmd(nc, in_maps, core_ids=list(range(N_CORES)))
    return assemble_output(res.results)
